# revision 59
# baseline (speedup 1.0000x reference)
"""HashEmbedding (hash -> gather -> sum-pool) on 8 TRN2 NeuronCores.

Strategy: batch-data-parallel (each core owns 512 of the 4096 batch rows
and a full copy of the [1M, 128] table in its local HBM). Per-core gather
traffic matches vocab-sharding but needs no collectives.

The gather primitive is the ANT `dma_gather` (gpsimd SWDGE, int16 indices
-> 31 fixed 32768-row window gathers, capacity-bounded). Perf history:
- baseline (single queue, f32): 1173us, GpSimd engine 96% busy at ~8.8us
  per 1024-index call -> Q7 descriptor generation bound.
- 4 SWDGE queues (each dma_gather runs on the Q7 core pair picked by
  queue_num, so 4 desc-gens run concurrently) + bf16 table and bf16
  pooling matmuls: 436us, GpSimd 86% / DMA engines ~80% busy.
- per-call static num_idxs = max bucket count across the 8 cores rounded
  up to 128 (the compile is specialized to the input's bucket histogram;
  ~12% fewer descriptors, matmuls, and assignment columns), packed chunk
  layout, single_packet=False so the SDMA engines interleave packets
  across the 4 queue rings (DMA busy fell ~3x: ~16ns/descriptor): 315us.
- whole-window calls (31 x ~3600 idxs) REGRESSED to 470us: the per-queue
  descriptor ring (dynamic_dma_scratch/32 descs) then holds only one
  call, so the NX decode stalls waiting for the previous same-queue
  call's DMA drain, and those stalls serialize all queues.
- 124 x ~900-idx calls round-robin across the 4 queues, ring doubled
  (scratch 65536 = 2048 descs/queue = 2+ calls in flight), gather tile
  ring 8 deep, bucket indices sorted ascending for HBM row locality:
  313-317us. The GpSimd chain (~283us) is the floor: a serial
  microbenchmark showed ~8.5us of Q7-pair time per 1024-idx call on
  EVERY queue (~8ns/index, unpack + descriptor-ring writes), so with
  4-way queue overlap the Pool engine is bound at aggregate_work/4.
- final: call sizes trimmed to exact 16-multiples of the max bucket
  count (n16, -3.1% descriptors/unpack vs 128-rounding; the [n16, n128)
  tail of each last chunk is pre-zeroed on the DVE): 310us.
  Variants that regressed: whole-window 3840-idx calls (ring holds only
  one call -> NX decode stalls on drain, 470us), 2048-idx pair calls
  (327us), packed 16B-aligned index layout instead of 128B-aligned
  64-col slots (slows the Q7 index read stream, +40% Q7 time),
  largest-first window order (+110us, cause unclear).

Pooling: per gathered chunk of 128 rows, a 0/1 assignment matrix
A[p, m] = (slot[p] == m) is built on the DVE via is_equal against an
iota, and psum[m, d] += A^T @ G accumulates the sum-pool in f32 PSUM.
Padding slots are -1 so they match no column and contribute zero.
"""

import sys

if "/opt/trn_rl_repo" not in sys.path:
    sys.path.insert(0, "/opt/trn_rl_repo")

import ml_dtypes
import numpy as np

B, H, D, V = 4096, 200, 128, 1_000_000
NCORES = 8
BPC = B // NCORES              # 512 batch rows per core
NPASS = 4                      # batch groups of 128 rows (PSUM M limit)
WBITS = 15
W = 1 << WBITS                 # 32768-row window (int16 index limit)
NW = (V + W - 1) // W          # 31 windows
CAP = 1024                     # hard capacity per (window, pass) bucket
CALL_IDX = NPASS * CAP         # flat index layout stride per window
CHUNKS = CALL_IDX // 128       # max matmul chunks per window
NQ = 4                         # SWDGE queues (Q7 core pairs)
GBUFS = 8                      # gather tile ring depth (2/queue)

_cache: dict = {}


def _bucket_counts(x_core):
    """Per-(window, pass) bucket histogram for one core. Also returns the
    (idx, b) decomposition reused by _host_prep."""
    idx = (
        (x_core.astype(np.uint32).ravel() * np.uint32(2654435761))
        % np.uint32(V)
    ).astype(np.int32)                       # [BPC*H]
    b = np.repeat(np.arange(BPC, dtype=np.int32), H)
    bucket = (idx >> WBITS) * NPASS + (b >> 7)
    counts = np.bincount(bucket, minlength=NW * NPASS)
    return idx, b, bucket, counts


def _host_prep(idx, b, bucket, n128):
    """Window-sort one core's positions -> (loc16 [NW,128,256] wrapped,
    slotf [NW,128,CHUNKS] bf16 with per-window used chunks packed
    contiguously). n128 [NW, NPASS]: static per-call index counts (>= this
    core's bucket counts); padding gathers row 0 with slot=-1 (matches no
    assignment column -> contributes zero)."""
    loc = idx & (W - 1)
    slot = b & 127

    # sort by (bucket, loc): ascending row order within each gather call
    # gives the SDMA engines / HBM banks much better access locality
    order = np.argsort(bucket.astype(np.int64) * W + loc, kind="stable")
    bs, ls, ss = bucket[order], loc[order], slot[order]
    counts = np.bincount(bucket, minlength=NW * NPASS)
    starts = np.zeros(NW * NPASS, dtype=np.int64)
    starts[1:] = np.cumsum(counts)[:-1]
    rank = np.arange(bs.size) - starts[bs]

    loc_arr = np.zeros((NW, NPASS, CAP), dtype=np.int16)
    slot_arr = np.full((NW, NPASS, CAP), -1.0, dtype=np.float32)
    loc_arr[bs // NPASS, bs % NPASS, rank] = ls.astype(np.int16)
    slot_arr[bs // NPASS, bs % NPASS, rank] = ss.astype(np.float32)

    # SWDGE wrapped layout: position i at [partition i%16, col i//16],
    # replicated to all 8 Q7-core partition groups (any queue's pair
    # reads the copy on its own partitions). Call (w, grp) reads cols
    # [grp*64, grp*64 + n128[w,grp]//16): keeping each call's slice at a
    # fixed 64-col slot keeps its SBUF address 128B-aligned — a packed
    # (16B-aligned) layout measurably slowed the Q7 index read stream.
    flat_loc = loc_arr.reshape(NW, CALL_IDX)
    wrapped = flat_loc.reshape(NW, CALL_IDX // 16, 16).transpose(0, 2, 1)
    loc16 = np.tile(wrapped, (1, 8, 1)).copy()            # [NW, 128, 256]

    # slot layout matching gather output (position i -> p=i%128, c=i//128),
    # with each window's used chunks packed contiguously:
    # col off[w,grp]+c holds call (w,grp) chunk c.
    cw = n128 // 128                                       # [NW, NPASS]
    slotf = np.full((NW, 128, CHUNKS), -1.0, dtype=np.float32)
    for w in range(NW):
        off = 0
        for g in range(NPASS):
            k = cw[w, g]
            chunks = slot_arr[w, g, : k * 128].reshape(k, 128).T  # [128, k]
            slotf[w, :, off : off + k] = chunks
            off += k
    return loc16, slotf.astype(ml_dtypes.bfloat16)


# HW-measured with a 12-calls-per-queue serial microbenchmark: a 1024-idx
# dma_gather call costs ~8.5us of Q7 pair time on EVERY queue (no per-queue
# asymmetry), so balanced round-robin assignment is optimal and the Pool
# engine floor is aggregate_work/4.


def _build(n128, n16):
    import concourse.tile as tile
    from concourse import bacc, mybir

    i16, i32 = mybir.dt.int16, mybir.dt.int32
    f32, bf16 = mybir.dt.float32, mybir.dt.bfloat16
    Alu = mybir.AluOpType

    cw = n128 // 128                   # [NW, NPASS] chunks per call
    tcw = cw.sum(axis=1)               # [NW] used chunks per window

    nc = bacc.Bacc(
        "TRN2",
        target_bir_lowering=False,
        debug=False,
        enable_asserts=False,
        # SWDGE descriptor carveout: ring capacity is scratch/32 descs per
        # queue (each queue pair's partitions hold its own rings). 65536
        # holds 2048 descs = two ~1024-desc calls in flight per queue, so
        # the NX decode never stalls waiting for the prior call's drain.
        dynamic_dma_scratch_size=65536,
        num_swdge_queues=NQ,
    )
    tb_ap = nc.dram_tensor("table", [NW * W, D], bf16, kind="ExternalInput").ap()
    loc_ap = nc.dram_tensor(
        "loc16", [NW, 128, CALL_IDX // 16], i16, kind="ExternalInput"
    ).ap()
    slot_ap = nc.dram_tensor(
        "slotf", [NW, 128, CHUNKS], bf16, kind="ExternalInput"
    ).ap()
    out_ap = nc.dram_tensor("out", [BPC, D], f32, kind="ExternalOutput").ap()

    with tile.TileContext(nc) as tc:
        with (
            tc.tile_pool(name="iop", bufs=1) as iop,
            tc.tile_pool(name="inp", bufs=4) as inp,
            tc.tile_pool(name="gp", bufs=GBUFS) as gp,
            tc.tile_pool(name="ap_", bufs=3) as ap_,
            tc.tile_pool(name="op", bufs=2) as op,
            tc.tile_pool(name="pp", bufs=1, space="PSUM") as pp,
        ):
            iota_i = iop.tile([128, 128], i32, name="iota_i")
            nc.gpsimd.iota(iota_i[:], [[1, 128]], base=0, channel_multiplier=0)
            iota_b = iop.tile([128, 128], bf16, name="iota_b")
            nc.vector.tensor_copy(iota_b[:], iota_i[:])

            psums = [
                pp.tile([128, D], f32, name=f"ps{g}", tag=f"ps{g}")
                for g in range(NPASS)
            ]

            qn = 0
            for wi, w in enumerate(range(NW)):
                lt = inp.tile([128, CALL_IDX // 16], i16, name="lt", tag="lt")
                nc.sync.dma_start(out=lt[:], in_=loc_ap[w])
                st = inp.tile([128, CHUNKS], bf16, name="st", tag="st")
                nc.sync.dma_start(out=st[:], in_=slot_ap[w])

                t = int(tcw[w])
                A = ap_.tile([128, t, 128], bf16, name="A", tag="A")
                iota_bc = iota_b[:].unsqueeze(1).broadcast_to([128, t, 128])
                st_bc = st[:, :t].unsqueeze(2).broadcast_to([128, t, 128])
                nc.vector.tensor_tensor(A[:], iota_bc, st_bc, Alu.is_equal)

                off = 0
                for grp in range(NPASS):
                    n = int(n16[w, grp])
                    k = int(cw[w, grp])
                    # one gather per (window, batch group); queue_num picks
                    # the Q7 core pair, so 4 desc-gens run concurrently
                    g = gp.tile([128, k, D], bf16, name="g", tag="g")
                    # pre-zero the last chunk: the gather only writes
                    # positions < n16, and the matmul must not read
                    # undefined data in the [n16, n128) tail (its
                    # assignment columns are -1 -> weight 0). Unconditional
                    # so every call consumes the same semaphore pattern
                    # (Tile's SWDGE sem lanes are locked per queue).
                    nc.vector.memset(g[:, k - 1 : k, :], 0.0)
                    nc.gpsimd.dma_gather(
                        g[:],
                        tb_ap[w * W : (w + 1) * W, :],
                        lt[:, grp * (CAP // 16) : grp * (CAP // 16) + n // 16],
                        n,
                        n,
                        D,
                        queue_num=qn % NQ,
                        single_packet=False,
                    )
                    qn += 1
                    for c in range(k):
                        nc.tensor.matmul(
                            psums[grp][:],
                            A[:, off + c, :],
                            g[:, c, :],
                            start=(wi == 0 and c == 0),
                            stop=(wi == NW - 1 and c == k - 1),
                        )
                    off += k

            for grp in range(NPASS):
                outs = op.tile([128, D], f32, name="outs", tag="outs")
                nc.vector.tensor_copy(outs[:], psums[grp][:])
                nc.sync.dma_start(
                    out=out_ap[grp * 128 : (grp + 1) * 128, :], in_=outs[:]
                )

    nc.compile()
    return nc


def _prep_inputs(x, table):
    x_np = np.asarray(x)
    per_core = [
        _bucket_counts(x_np[c * BPC : (c + 1) * BPC]) for c in range(NCORES)
    ]
    counts_max = np.max([pc[3] for pc in per_core], axis=0)
    if counts_max.max() > CAP:
        raise RuntimeError(f"window bucket overflow: {counts_max.max()} > {CAP}")
    n128 = (
        ((counts_max.reshape(NW, NPASS) + 127) // 128) * 128
    ).astype(np.int64)

    # exact 16-multiple call sizes: descriptors/unpack stop at n16 while
    # the chunk layout stays n128-padded (the [n16, n128) tail of the last
    # chunk is zeroed on-device after each gather)
    n16 = (((counts_max.reshape(NW, NPASS) + 15) // 16) * 16).astype(np.int64)

    # pad the table to NW*W rows so every gather window is a full 32768
    tb = np.zeros((NW * W, D), dtype=ml_dtypes.bfloat16)
    tb[:V] = np.asarray(table).astype(ml_dtypes.bfloat16)
    in_maps = []
    for c in range(NCORES):
        idx, b, bucket, _ = per_core[c]
        loc16, slotf = _host_prep(idx, b, bucket, n128)
        in_maps.append({"table": tb, "loc16": loc16, "slotf": slotf})
    return n128, n16, in_maps


def _run(x, table, trace=False):
    from concourse.bass_utils import run_bass_kernel_spmd

    n128, n16, in_maps = _prep_inputs(x, table)
    key = n128.tobytes() + n16.tobytes()
    if _cache.get("key") != key:
        _cache["nc"] = _build(n128, n16)
        _cache["key"] = key
    nc = _cache["nc"]

    res = run_bass_kernel_spmd(nc, in_maps, list(range(NCORES)), trace=trace)
    out = np.concatenate(
        [res.results[c]["out"] for c in range(NCORES)], axis=0
    ).astype(np.float32)
    return out, res


def kernel(x, table):
    out, _ = _run(x, table, trace=False)
    return out
